# revision 2
# baseline (speedup 1.0000x reference)
"""Causal self-attention (B=4, T=2048, C=1024, H=16, D=64) on 8 TRN2 cores, v2.

Sharding: core = 2*b + hg  (b = batch 0..3, hg = head-group 0..1 of 8 heads).
Host passes x^T and the wq/wk/wv/wproj slices transposed; weights pre-scaled
x64 so fp8e4 uses its normal range (1/64 folded into the v-blend scalar and
the output scale; q/k need no correction since RMS norm is scale invariant).

Per-core pipeline:
  p1 (per 128-token tile): QKV via fp8 DoubleRow matmuls; RMS stats + scale
     into fp16; RoPE in fp16 on DVE (2x mode); 4 PE transposes batched into
     one PSUM bank (shared zero-region start/stop) + one wide DVE copy into
     feature-major qT/kT; v blended into [128,8,65] fp16 tiles whose col 64
     is 1.0.
  p2 (per 512-query chunk, per head): S^T = kT.T @ qT fp16 into PSUM pairs
     [128,2,512]; exp on ACT batched per pair (diagonal pairs windowed + fp16
     triangle multiply); flipped PV (out[128 tokens, 65] per head, pt
     stationary, v moving) accumulating softmax denominators via the ones
     column; gpsimd normalize_recip divides per partition; y transposed back
     to feature-major fp8 (batched-bank + one copy); fp8 DoubleRow output
     projection; fp16 store.

The ISA has ONE semaphore-wait slot per instruction; Tile emits more.
_legalize_waits() splits extras onto same-engine NoOps post-scheduling.
"""

import math

import numpy as np
import ml_dtypes

import concourse.bass as bass
import concourse.mybir as mybir
import concourse.tile as tile
from concourse import bass_utils
from concourse.masks import make_identity

F32 = mybir.dt.float32
F16 = mybir.dt.float16
I16 = mybir.dt.int16
F8 = mybir.dt.float8e4
NPF8 = ml_dtypes.float8_e4m3
DR = mybir.MatmulPerfMode.DoubleRow

B, T, C, H, D = 4, 2048, 1024, 16, 64
HG = C // 2          # 512 features per head group (8 heads x 64)
NT = T // 128        # 16 t-tiles
NQ = T // 512        # 4 query chunks
EPS = 1.1920928955078125e-07
SCALE = 1.0 / math.sqrt(D)  # 0.125
WSCALE = 1.0         # no weight prescale needed at fp16
# Schraudolph fast-exp (f16 bit trick) for the first N below-diagonal pairs
# of every (chunk, head): exp(SCALE*s) ~= bitcast_f16(int16(A*s + B))
N_SCHRAUDOLPH = 2
SCHED = 0
WARM_TILES = 4
OSB_DVE = True
QK_COPY_DVE = False
SCH_Q2 = 2
ST_BUFS = 4
YU_RING = False
YU_BUFS = 1
SCH_A = 1024.0 / math.log(2.0) * SCALE
SCH_B = 15360.0 - 59.27

_wsplit_counter = [0]


def _legalize_waits(nc):
    """Split multi-wait instructions into single-wait NoOp chains."""
    n = 0
    for f in nc.m.functions:
        for bb in f.blocks:
            new_list = []
            changed = False
            for inst in bb.instructions:
                si = inst.sync_info
                if si is not None and si.on_wait and len(si.on_wait) > 1:
                    waits = list(si.on_wait)
                    for w in waits[:-1]:
                        _wsplit_counter[0] += 1
                        new_list.append(mybir.InstNoOp(
                            name=f"WSPLIT-{_wsplit_counter[0]}",
                            engine=inst.engine, ins=[], outs=[],
                            sync_info=mybir.SyncInfo(on_wait=[w], on_update=[]),
                        ))
                    si.on_wait = waits[-1:]
                    changed = True
                    n += 1
                new_list.append(inst)
            if changed:
                bb.instructions = new_list
    return n


def _build(lam: float) -> bass.Bass:
    nc = bass.Bass("TRN2", target_bir_lowering=False, debug=False, num_devices=8)

    xb_d = nc.dram_tensor("xbT", [C, T], F16, kind="ExternalInput").ap()
    v1_d = nc.dram_tensor("v1b", [T, HG], F16, kind="ExternalInput").ap()
    wq_d = nc.dram_tensor("wqT", [C, HG], F16, kind="ExternalInput").ap()
    wk_d = nc.dram_tensor("wkT", [C, HG], F16, kind="ExternalInput").ap()
    wv_d = nc.dram_tensor("wvT", [C, HG], F16, kind="ExternalInput").ap()
    wp_d = nc.dram_tensor("wpT", [HG, C], F16, kind="ExternalInput").ap()
    cos_d = nc.dram_tensor("cosn", [T, 32], F16, kind="ExternalInput").ap()
    sin_d = nc.dram_tensor("sinn", [T, 64], F16, kind="ExternalInput").ap()
    tri_d = nc.dram_tensor("tri01", [128, 128], F16, kind="ExternalInput").ap()
    out_d = nc.dram_tensor("out", [T, C], F16, kind="ExternalOutput").ap()

    with tile.TileContext(nc) as tc:
        with (
            tc.tile_pool(name="const", bufs=1) as const,
            tc.tile_pool(name="pers", bufs=1) as pers,
        ):
            identf = const.tile([128, 128], F32)
            make_identity(nc, identf)
            ident = const.tile([128, 128], F16)
            nc.scalar.copy(out=ident, in_=identf)
            tri16 = const.tile([128, 128], F16)
            nc.gpsimd.dma_start(out=tri16, in_=tri_d)
            epsc = const.tile([128, 1], F32)
            nc.vector.memset(epsc, EPS)

            # persistent feature-major q/k (fp16; dim1 = head-pair j) and v
            # tiles (fp16, col 64 = 1.0 accumulates softmax denominators)
            qTa = pers.tile([128, 4, T], F16, name="qTa", tag="qTa")
            kTa = pers.tile([128, 4, T], F16, name="kTa", tag="kTa")
            vsb = [pers.tile([128, 8, 65], F16, name=f"v{t}", tag=f"v{t}")
                   for t in range(NT)]
            for t in range(NT):
                nc.gpsimd.memset(vsb[t][:, :, 64:65], 1.0)

            with (
                tc.tile_pool(name="p1", bufs=1) as p1,
                tc.tile_pool(name="ring", bufs=1, space="PSUM") as ringp,
                tc.tile_pool(name="stp", bufs=1, space="PSUM") as stpp,
                tc.tile_pool(name="yup", bufs=1, space="PSUM") as yupp,
                tc.tile_pool(name="tpp", bufs=1, space="PSUM") as tppp,
                tc.tile_pool(name="p2", bufs=1) as p2,
            ):
                wq_sb = p1.tile([128, 8, HG], F16)
                wk_sb = p1.tile([128, 8, HG], F16)
                wv_sb = p1.tile([128, 8, HG], F16)
                wp_sb = p1.tile([128, 4, C], F16)
                cos_sb = p1.tile([128, NT, 32], F16)
                sin_sb = p1.tile([128, NT, 64], F16)

                def dma_weights_early():
                    nc.sync.dma_start(
                        out=wq_sb,
                        in_=wq_d.rearrange("(c p) i -> p c i", p=128))
                    nc.sync.dma_start(
                        out=cos_sb,
                        in_=cos_d.rearrange("(n p) i -> p n i", p=128))
                    nc.sync.dma_start(
                        out=sin_sb,
                        in_=sin_d.rearrange("(n p) i -> p n i", p=128))

                def dma_weights_late():
                    nc.sync.dma_start(
                        out=wk_sb,
                        in_=wk_d.rearrange("(c p) i -> p c i", p=128))
                    nc.sync.dma_start(
                        out=wv_sb,
                        in_=wv_d.rearrange("(c p) i -> p c i", p=128))
                    nc.sync.dma_start(
                        out=wp_sb,
                        in_=wp_d.rearrange("(c p) j -> p c j", p=128))

                w_sb = {"q": wq_sb, "k": wk_sb, "v": wv_sb}

                def trans4(src, dst_ap, fast_copy):
                    """4 batched [128,128] transposes of fp16 `src` through one
                    PSUM bank, then a single wide copy to dst_ap ([128,4,128]).
                    """
                    tp = tppp.tile([128, 4, 128], F16, name="tp", tag="tp",
                                   bufs=1)
                    for j in range(4):
                        nc.tensor.matmul(
                            tp[:, j, :], src[:, j * 128:(j + 1) * 128], ident,
                            start=(j == 0), stop=(j == 3), is_transpose=True,
                            skip_group_check=True)
                    if fast_copy:
                        nc.vector.tensor_copy(out=dst_ap, in_=tp)
                    else:
                        nc.scalar.copy(out=dst_ap, in_=tp)

                def p1_units(tc4):
                    """Phase-1 closures for chunk tc4: dma, q*4, k*4, v*4."""
                    xT = p1.tile([128, 8, 512], F16, name="xT", tag="xT",
                                 bufs=2)
                    t0 = tc4 * 512

                    def dma_x():
                        nc.sync.dma_start(
                            out=xT,
                            in_=xb_d[:, t0:t0 + 512].rearrange(
                                "(c p) t -> p c t", p=128))
                    units = [dma_x]
                    for which in ("q", "k", "v"):
                        for ts in range(4):
                            tg = tc4 * 4 + ts
                            units.append(lambda ts=ts, tg=tg, which=which: (
                                p1_unit(xT, ts, tg, which)))
                    return units

                def p1_unit(xT, ts, tg, which):
                    ps = ringp.tile([128, 512], F32, name="ps", tag="ring",
                                    bufs=2)
                    for cc in range(8):
                        nc.tensor.matmul(
                            ps,
                            xT[:, cc, ts * 128:(ts + 1) * 128],
                            w_sb[which][:, cc, :],
                            start=(cc == 0), stop=(cc == 7))
                    p3 = ps.rearrange("p (h d) -> p h d", h=8)
                    if which == "v":
                        v1t = p1.tile([128, HG], F16, name="v1t",
                                      tag="v1t", bufs=2)
                        nc.sync.dma_start(
                            out=v1t, in_=v1_d[tg * 128:(tg + 1) * 128, :])
                        # v = (1-lam)/64 * (x@wv') + lam*v1 (v1 pre-scaled)
                        nc.vector.scalar_tensor_tensor(
                            out=vsb[tg][:, :, 0:64],
                            in0=p3,
                            scalar=(1.0 - lam),
                            in1=v1t.rearrange("p (h d) -> p h d", h=8),
                            op0=mybir.AluOpType.mult,
                            op1=mybir.AluOpType.add)
                        return

                    # ---- copy psum to fp16 once (frees the PSUM ring
                    # fast; everything downstream runs in DVE 2x mode) ----
                    warm = tg < WARM_TILES  # chunk 0: ACT is idle, help out
                    qq = p1.tile([128, 512], F16, name="qq", tag="qq", bufs=3)
                    qq3 = qq.rearrange("p (h d) -> p h d", h=8)
                    if warm:
                        nc.scalar.copy(out=qq, in_=ps)
                    else:
                        nc.vector.tensor_copy(out=qq, in_=ps)
                    # ---- RMS stats (q/k) ----
                    sq = p1.tile([128, 512], F16, name="sq", tag="sq", bufs=2)
                    if warm:
                        nc.scalar.square(out=sq, in_=ps)
                    else:
                        nc.vector.tensor_mul(out=sq, in0=qq, in1=qq)
                    ssum = p1.tile([128, 8], F16, name="ssum", tag="ssum",
                                   bufs=4)
                    with nc.allow_low_precision(reason="rms sumsq in f16"):
                        nc.vector.tensor_reduce(
                            ssum, sq.rearrange("p (h d) -> p h d", h=8),
                            axis=mybir.AxisListType.X, op=mybir.AluOpType.add)
                    srt = p1.tile([128, 8], F32, name="srt", tag="srt", bufs=4)
                    nc.scalar.activation(
                        srt, ssum, mybir.ActivationFunctionType.Sqrt,
                        bias=epsc, scale=1.0 / 64.0)
                    rst = p1.tile([128, 8], F16, name="rst", tag="rst", bufs=4)
                    with nc.allow_low_precision(reason="rms recip in f16"):
                        nc.vector.reciprocal(out=rst, in_=srt)
                    # qn = qq * rst (broadcast over d), all-fp16 2x
                    qn = p1.tile([128, 512], F16, name="qn", tag="qn", bufs=2)
                    qn3 = qn.rearrange("p (h d) -> p h d", h=8)
                    rstb = rst.rearrange("p (h o) -> p h o", o=1).to_broadcast(
                        (128, 8, 64))
                    nc.vector.tensor_mul(out=qn3, in0=qq3, in1=rstb)

                    # ---- RoPE in fp16 (DVE 2x): tm = swap(x)*[s|-s]; then
                    # y = x*[c|c] + tm  (2 half-width muls + full mul + add)
                    rot = p1.tile([128, 512], F16, name="rot", tag="rot",
                                  bufs=4)
                    r4 = rot.rearrange("p (h j d) -> p h j d", h=8, j=2)
                    tm = p1.tile([128, 512], F16, name="tm", tag="tm", bufs=2)
                    tm4 = tm.rearrange("p (h j d) -> p h j d", h=8, j=2)
                    c4 = cos_sb[:, tg, :].rearrange(
                        "p (o u i) -> p o u i", o=1, u=1).to_broadcast(
                        (128, 8, 2, 32))
                    s4 = sin_sb[:, tg, :].rearrange(
                        "p (h j d) -> p h j d", h=1, j=2).to_broadcast(
                        (128, 8, 2, 32))
                    qn4 = qn.rearrange("p (h j d) -> p h j d", h=8, j=2)
                    nc.vector.tensor_mul(out=tm4[:, :, 0, :],
                                         in0=qn4[:, :, 1, :],
                                         in1=s4[:, :, 0, :])
                    nc.vector.tensor_mul(out=tm4[:, :, 1, :],
                                         in0=qn4[:, :, 0, :],
                                         in1=s4[:, :, 1, :])
                    nc.vector.tensor_mul(out=r4, in0=qn4, in1=c4)
                    nc.vector.tensor_add(out=r4, in0=r4, in1=tm4)

                    # ---- transpose to feature-major (one bank, one copy) ----
                    dstT = qTa if which == "q" else kTa
                    trans4(rot, dstT[:, :, tg * 128:(tg + 1) * 128],
                           fast_copy=QK_COPY_DVE)

                def pv_head_parcels(qc, h, ptq, ynorm):
                    """Flipped PV for one head as 5 parcels (4 subs +
                    normalize) so they can interleave with S pairs."""
                    yu_box = []

                    def sub_parcel(sub):
                        if sub == 0:
                            yu_box.append((ringp if YU_RING else yupp).tile(
                                [128, 512], F32, name="yu",
                                tag="ring" if YU_RING else "yu", bufs=YU_BUFS))
                        yu3 = yu_box[0].rearrange(
                            "p (s f) -> p s f", s=4)[:, :, 0:65]
                        last_kt = 4 * qc + sub
                        for kt in range(last_kt + 1):
                            nc.tensor.matmul(
                                yu3[:, sub, :],
                                ptq[kt // 2][:, kt % 2,
                                             sub * 128:(sub + 1) * 128],
                                vsb[kt][:, h, :],
                                start=(sub == 0 and kt == 0),
                                stop=(sub == 3 and kt == last_kt),
                                skip_group_check=True)

                    def norm_parcel():
                        yu3 = yu_box[0].rearrange(
                            "p (s f) -> p s f", s=4)[:, :, 0:65]
                        rec = p2.tile([128, 4, 1], F32, name="rec", tag="rec",
                                      bufs=2)
                        nc.vector.reciprocal(out=rec, in_=yu3[:, :, 64:65])
                        nc.vector.tensor_mul(
                            out=ynorm[:, :, h, :], in0=yu3[:, :, 0:64],
                            in1=rec.to_broadcast((128, 4, 64)))

                    return [lambda s=s: sub_parcel(s) for s in range(4)] + \
                        [norm_parcel]

                def do_p2(qc, units, pre_pv):
                    """Attention for query chunk qc; `units` are next-chunk
                    phase-1 closures woven between heads; the first `pre_pv`
                    units are emitted before head 0's PV (qc==0 needs chunk-0
                    v tiles there).  PV runs one head behind S/exp so the PE
                    never waits on the current head's exp."""
                    npair = 2 * qc + 2
                    it = iter(units)
                    ynorm = p2.tile([128, 4, 8, 64], F16, name="yn",
                                    tag="yn", bufs=2)
                    prev = None
                    for h in range(8):
                        hp, b0 = h // 2, 64 * (h % 2)
                        # fillers: previous head's PV parcels + next-chunk
                        # phase-1 units, spread between this head's S pairs
                        fillers = []
                        if prev is not None:
                            fillers.extend(
                                pv_head_parcels(qc, prev[0], prev[1], ynorm))
                        for _ in range(pre_pv if h == 0 else 2):
                            u = next(it, None)
                            if u is not None:
                                fillers.append(u)
                        fidx = [0.0]
                        fstep = len(fillers) / (4 * qc + 4) if SCHED == 2 else 0.0

                        def drain(upto):
                            while fidx[0] < upto and int(fidx[0]) < len(fillers):
                                fillers[int(fidx[0])]()
                                fidx[0] += 1.0

                        if SCHED == 1:
                            drain(len(fillers))
                        ptq = [p2.tile([128, 2, 512], F16, name="ptq",
                                       tag="ptq", bufs=4 * NQ + 2)
                               for _ in range(npair)]
                        for kt in range(4 * qc + 4):
                            m = kt - 4 * qc
                            a0 = max(0, 128 * m)
                            st1 = stpp.tile([128, 512], F32, name="st1",
                                            tag="st1", bufs=ST_BUFS)
                            nc.tensor.matmul(
                                st1[:, a0:512],
                                kTa[b0:b0 + 64, hp,
                                    kt * 128:(kt + 1) * 128],
                                qTa[b0:b0 + 64, hp,
                                    qc * 512 + a0:(qc + 1) * 512],
                                start=True, stop=True)
                            ptt = ptq[kt // 2][:, kt % 2, :]
                            if m < 0 and qc >= 2 and kt < (2 * SCH_Q2 if qc == 2 else 2 * N_SCHRAUDOLPH):
                                # fast exp: f16 bits = int16(A*s + B)
                                nc.vector.tensor_scalar(
                                    out=ptt.bitcast(I16), in0=st1,
                                    scalar1=SCH_A, scalar2=SCH_B,
                                    op0=mybir.AluOpType.mult,
                                    op1=mybir.AluOpType.add)
                            elif m < 0:
                                nc.scalar.activation(
                                    ptt, st1,
                                    mybir.ActivationFunctionType.Exp,
                                    scale=SCALE)
                            else:
                                w0 = 128 * m
                                nc.scalar.activation(
                                    ptt[:, w0:512], st1[:, w0:512],
                                    mybir.ActivationFunctionType.Exp,
                                    scale=SCALE)
                                nc.gpsimd.tensor_mul(
                                    out=ptt[:, w0:w0 + 128],
                                    in0=ptt[:, w0:w0 + 128],
                                    in1=tri16)
                            if SCHED == 2:
                                drain((kt + 1) * fstep)

                        drain(len(fillers))
                        prev = (h, ptq)

                    for f in pv_head_parcels(qc, prev[0], prev[1], ynorm):
                        f()

                    # ---- y -> feature-major fp8; projection; store ----
                    yT8 = p2.tile([128, 4, 512], F16, name="yT8", tag="yT8",
                                  bufs=2)
                    for sub in range(4):
                        u = next(it, None)
                        if u is not None:
                            u()
                        yflat = ynorm[:, sub, :, :].rearrange(
                            "p h d -> p (h d)")
                        trans4(yflat, yT8[:, :, sub * 128:(sub + 1) * 128],
                               fast_copy=True)
                        for jc in range(2):
                            pr = ringp.tile([128, 512], F32, name="pr",
                                            tag="ring", bufs=2)
                            for i in range(4):
                                nc.tensor.matmul(
                                    pr,
                                    yT8[:, i, sub * 128:(sub + 1) * 128],
                                    wp_sb[:, i, jc * 512:(jc + 1) * 512],
                                    start=(i == 0), stop=(i == 3))
                            osb = p2.tile([128, 512], F16, name="osb",
                                          tag="osb", bufs=3)
                            if OSB_DVE and jc == 0:
                                nc.vector.tensor_copy(out=osb, in_=pr)
                            else:
                                nc.scalar.copy(out=osb, in_=pr)
                            r0 = qc * 512 + sub * 128
                            nc.sync.dma_start(
                                out=out_d[r0:r0 + 128,
                                          jc * 512:(jc + 1) * 512],
                                in_=osb)
                    for u in it:
                        u()

                for ii in range(NQ):
                    units = p1_units(ii)
                    if ii == 0:
                        units[0]()                 # x chunk-0 dma first
                        dma_weights_early()        # wq + rope tables
                        units[1]()                 # q ts0
                        dma_weights_late()         # wk, wv, wp
                        for u in units[2:9]:       # q*3 + k*4
                            u()
                        carry = units[9:]          # v*4
                    else:
                        do_p2(ii - 1, carry + units, pre_pv=len(carry))
                        carry = []
                do_p2(NQ - 1, [], pre_pv=0)

    _legalize_waits(nc)
    return nc


def _host_tables():
    inv_freq = 1.0 / (10000.0 ** (np.arange(0, D, 2, dtype=np.float32) / D))
    t = np.arange(T, dtype=np.float32)
    freqs = np.outer(t, inv_freq).astype(np.float32)      # (T, 32)
    cos16 = np.cos(freqs).astype(np.float16)
    s = np.sin(freqs)
    sin16 = np.concatenate([s, -s], axis=1).astype(np.float16)  # [T, 64]
    p = np.arange(128)[:, None]
    f = np.arange(128)[None, :]
    tri = (p <= f).astype(np.float16)                     # (128, 128)
    return cos16, sin16, tri


_CACHE = {}


def kernel(x, v1, wq, wk, wv, wproj, lamb):
    x = np.asarray(x, dtype=np.float32)
    v1 = np.asarray(v1, dtype=np.float32)
    wq = np.asarray(wq, dtype=np.float32)
    wk = np.asarray(wk, dtype=np.float32)
    wv = np.asarray(wv, dtype=np.float32)
    wproj = np.asarray(wproj, dtype=np.float32)
    lam = float(np.asarray(lamb))

    cosn, sinn, tri = _host_tables()

    key = lam
    if key not in _CACHE:
        _CACHE[key] = _build(lam)
    nc = _CACHE[key]

    in_maps = []
    for core in range(8):
        b, hg = core // 2, core % 2
        sl = slice(hg * HG, (hg + 1) * HG)
        in_maps.append({
            "xbT": np.ascontiguousarray(x[b].T).astype(np.float16),
            "v1b": np.ascontiguousarray(lam * v1[b][:, sl]).astype(np.float16),
            "wqT": np.ascontiguousarray(wq[sl, :].T).astype(np.float16),
            "wkT": np.ascontiguousarray(wk[sl, :].T).astype(np.float16),
            "wvT": np.ascontiguousarray(wv[sl, :].T).astype(np.float16),
            "wpT": np.ascontiguousarray(wproj[:, sl].T).astype(np.float16),
            "cosn": cosn,
            "sinn": sinn,
            "tri01": tri,
        })

    res = bass_utils.run_bass_kernel_spmd(nc, in_maps, core_ids=list(range(8)))
    y = np.empty((B, T, C), dtype=np.float32)
    for b in range(B):
        y[b] = (res.results[2 * b]["out"].astype(np.float32)
                + res.results[2 * b + 1]["out"].astype(np.float32))
    return (y, v1)


# revision 3
# speedup vs baseline: 1.0086x; 1.0086x over previous
"""Causal self-attention (B=4, T=2048, C=1024, H=16, D=64) on 8 TRN2 cores, v2.

Sharding: core = 2*b + hg  (b = batch 0..3, hg = head-group 0..1 of 8 heads).
Host passes x^T and the wq/wk/wv/wproj slices transposed; weights pre-scaled
x64 so fp8e4 uses its normal range (1/64 folded into the v-blend scalar and
the output scale; q/k need no correction since RMS norm is scale invariant).

Per-core pipeline:
  p1 (per 128-token tile): QKV via fp8 DoubleRow matmuls; RMS stats + scale
     into fp16; RoPE in fp16 on DVE (2x mode); 4 PE transposes batched into
     one PSUM bank (shared zero-region start/stop) + one wide DVE copy into
     feature-major qT/kT; v blended into [128,8,65] fp16 tiles whose col 64
     is 1.0.
  p2 (per 512-query chunk, per head): S^T = kT.T @ qT fp16 into PSUM pairs
     [128,2,512]; exp on ACT batched per pair (diagonal pairs windowed + fp16
     triangle multiply); flipped PV (out[128 tokens, 65] per head, pt
     stationary, v moving) accumulating softmax denominators via the ones
     column; gpsimd normalize_recip divides per partition; y transposed back
     to feature-major fp8 (batched-bank + one copy); fp8 DoubleRow output
     projection; fp16 store.

The ISA has ONE semaphore-wait slot per instruction; Tile emits more.
_legalize_waits() splits extras onto same-engine NoOps post-scheduling.
"""

import math

import numpy as np
import ml_dtypes

import concourse.bass as bass
import concourse.mybir as mybir
import concourse.tile as tile
from concourse import bass_utils
from concourse.masks import make_identity

F32 = mybir.dt.float32
F16 = mybir.dt.float16
I16 = mybir.dt.int16
F8 = mybir.dt.float8e4
NPF8 = ml_dtypes.float8_e4m3
DR = mybir.MatmulPerfMode.DoubleRow

B, T, C, H, D = 4, 2048, 1024, 16, 64
HG = C // 2          # 512 features per head group (8 heads x 64)
NT = T // 128        # 16 t-tiles
NQ = T // 512        # 4 query chunks
EPS = 1.1920928955078125e-07
SCALE = 1.0 / math.sqrt(D)  # 0.125
WSCALE = 1.0         # no weight prescale needed at fp16
# Schraudolph fast-exp (f16 bit trick) for the first N below-diagonal pairs
# of every (chunk, head): exp(SCALE*s) ~= bitcast_f16(int16(A*s + B))
N_SCHRAUDOLPH = 2
SCHED = 0
WARM_TILES = 6
OSB_DVE = True
QK_COPY_DVE = False
SCH_Q2 = 2
SCH_Q3 = 4
ST_BUFS = 5
YU_RING = True
YU_BUFS = 2
SCH_A = 1024.0 / math.log(2.0) * SCALE
SCH_B = 15360.0 - 59.27

_wsplit_counter = [0]


def _legalize_waits(nc):
    """Split multi-wait instructions into single-wait NoOp chains."""
    n = 0
    for f in nc.m.functions:
        for bb in f.blocks:
            new_list = []
            changed = False
            for inst in bb.instructions:
                si = inst.sync_info
                if si is not None and si.on_wait and len(si.on_wait) > 1:
                    waits = list(si.on_wait)
                    for w in waits[:-1]:
                        _wsplit_counter[0] += 1
                        new_list.append(mybir.InstNoOp(
                            name=f"WSPLIT-{_wsplit_counter[0]}",
                            engine=inst.engine, ins=[], outs=[],
                            sync_info=mybir.SyncInfo(on_wait=[w], on_update=[]),
                        ))
                    si.on_wait = waits[-1:]
                    changed = True
                    n += 1
                new_list.append(inst)
            if changed:
                bb.instructions = new_list
    return n


def _build(lam: float) -> bass.Bass:
    nc = bass.Bass("TRN2", target_bir_lowering=False, debug=False, num_devices=8)

    xb_d = nc.dram_tensor("xbT", [C, T], F16, kind="ExternalInput").ap()
    v1_d = nc.dram_tensor("v1b", [T, HG], F16, kind="ExternalInput").ap()
    wq_d = nc.dram_tensor("wqT", [C, HG], F16, kind="ExternalInput").ap()
    wk_d = nc.dram_tensor("wkT", [C, HG], F16, kind="ExternalInput").ap()
    wv_d = nc.dram_tensor("wvT", [C, HG], F16, kind="ExternalInput").ap()
    wp_d = nc.dram_tensor("wpT", [HG, C], F16, kind="ExternalInput").ap()
    cos_d = nc.dram_tensor("cosn", [T, 32], F16, kind="ExternalInput").ap()
    sin_d = nc.dram_tensor("sinn", [T, 64], F16, kind="ExternalInput").ap()
    tri_d = nc.dram_tensor("tri01", [128, 128], F16, kind="ExternalInput").ap()
    out_d = nc.dram_tensor("out", [T, C], F16, kind="ExternalOutput").ap()

    with tile.TileContext(nc) as tc:
        with (
            tc.tile_pool(name="const", bufs=1) as const,
            tc.tile_pool(name="pers", bufs=1) as pers,
        ):
            identf = const.tile([128, 128], F32)
            make_identity(nc, identf)
            ident = const.tile([128, 128], F16)
            nc.scalar.copy(out=ident, in_=identf)
            tri16 = const.tile([128, 128], F16)
            nc.gpsimd.dma_start(out=tri16, in_=tri_d)
            epsc = const.tile([128, 1], F32)
            nc.vector.memset(epsc, EPS)

            # persistent feature-major q/k (fp16; dim1 = head-pair j) and v
            # tiles (fp16, col 64 = 1.0 accumulates softmax denominators)
            qTa = pers.tile([128, 4, T], F16, name="qTa", tag="qTa")
            kTa = pers.tile([128, 4, T], F16, name="kTa", tag="kTa")
            vsb = [pers.tile([128, 8, 65], F16, name=f"v{t}", tag=f"v{t}")
                   for t in range(NT)]
            for t in range(NT):
                nc.gpsimd.memset(vsb[t][:, :, 64:65], 1.0)

            with (
                tc.tile_pool(name="p1", bufs=1) as p1,
                tc.tile_pool(name="ring", bufs=1, space="PSUM") as ringp,
                tc.tile_pool(name="stp", bufs=1, space="PSUM") as stpp,
                tc.tile_pool(name="yup", bufs=1, space="PSUM") as yupp,
                tc.tile_pool(name="tpp", bufs=1, space="PSUM") as tppp,
                tc.tile_pool(name="p2", bufs=1) as p2,
            ):
                wq_sb = p1.tile([128, 8, HG], F16)
                wk_sb = p1.tile([128, 8, HG], F16)
                wv_sb = p1.tile([128, 8, HG], F16)
                wp_sb = p1.tile([128, 4, C], F16)
                cos_sb = p1.tile([128, NT, 32], F16)
                sin_sb = p1.tile([128, NT, 64], F16)

                def dma_weights_early():
                    nc.sync.dma_start(
                        out=wq_sb,
                        in_=wq_d.rearrange("(c p) i -> p c i", p=128))
                    nc.sync.dma_start(
                        out=cos_sb,
                        in_=cos_d.rearrange("(n p) i -> p n i", p=128))
                    nc.sync.dma_start(
                        out=sin_sb,
                        in_=sin_d.rearrange("(n p) i -> p n i", p=128))

                def dma_weights_late():
                    nc.sync.dma_start(
                        out=wk_sb,
                        in_=wk_d.rearrange("(c p) i -> p c i", p=128))
                    nc.sync.dma_start(
                        out=wv_sb,
                        in_=wv_d.rearrange("(c p) i -> p c i", p=128))
                    nc.sync.dma_start(
                        out=wp_sb,
                        in_=wp_d.rearrange("(c p) j -> p c j", p=128))

                w_sb = {"q": wq_sb, "k": wk_sb, "v": wv_sb}

                def trans4(src, dst_ap, fast_copy):
                    """4 batched [128,128] transposes of fp16 `src` through one
                    PSUM bank, then a single wide copy to dst_ap ([128,4,128]).
                    """
                    tp = tppp.tile([128, 4, 128], F16, name="tp", tag="tp",
                                   bufs=1)
                    for j in range(4):
                        nc.tensor.matmul(
                            tp[:, j, :], src[:, j * 128:(j + 1) * 128], ident,
                            start=(j == 0), stop=(j == 3), is_transpose=True,
                            skip_group_check=True)
                    if fast_copy:
                        nc.vector.tensor_copy(out=dst_ap, in_=tp)
                    else:
                        nc.scalar.copy(out=dst_ap, in_=tp)

                def p1_units(tc4):
                    """Phase-1 closures for chunk tc4: dma, q*4, k*4, v*4."""
                    xT = p1.tile([128, 8, 512], F16, name="xT", tag="xT",
                                 bufs=2)
                    t0 = tc4 * 512

                    def dma_x():
                        nc.sync.dma_start(
                            out=xT,
                            in_=xb_d[:, t0:t0 + 512].rearrange(
                                "(c p) t -> p c t", p=128))
                    units = [dma_x]
                    for which in ("q", "k", "v"):
                        for ts in range(4):
                            tg = tc4 * 4 + ts
                            units.append(lambda ts=ts, tg=tg, which=which: (
                                p1_unit(xT, ts, tg, which)))
                    return units

                def p1_unit(xT, ts, tg, which):
                    ps = ringp.tile([128, 512], F32, name="ps", tag="ring",
                                    bufs=2)
                    for cc in range(8):
                        nc.tensor.matmul(
                            ps,
                            xT[:, cc, ts * 128:(ts + 1) * 128],
                            w_sb[which][:, cc, :],
                            start=(cc == 0), stop=(cc == 7))
                    p3 = ps.rearrange("p (h d) -> p h d", h=8)
                    if which == "v":
                        v1t = p1.tile([128, HG], F16, name="v1t",
                                      tag="v1t", bufs=2)
                        nc.sync.dma_start(
                            out=v1t, in_=v1_d[tg * 128:(tg + 1) * 128, :])
                        # v = (1-lam)/64 * (x@wv') + lam*v1 (v1 pre-scaled)
                        nc.vector.scalar_tensor_tensor(
                            out=vsb[tg][:, :, 0:64],
                            in0=p3,
                            scalar=(1.0 - lam),
                            in1=v1t.rearrange("p (h d) -> p h d", h=8),
                            op0=mybir.AluOpType.mult,
                            op1=mybir.AluOpType.add)
                        return

                    # ---- copy psum to fp16 once (frees the PSUM ring
                    # fast; everything downstream runs in DVE 2x mode) ----
                    warm = tg < WARM_TILES  # chunk 0: ACT is idle, help out
                    qq = p1.tile([128, 512], F16, name="qq", tag="qq", bufs=3)
                    qq3 = qq.rearrange("p (h d) -> p h d", h=8)
                    if warm:
                        nc.scalar.copy(out=qq, in_=ps)
                    else:
                        nc.vector.tensor_copy(out=qq, in_=ps)
                    # ---- RMS stats (q/k) ----
                    sq = p1.tile([128, 512], F16, name="sq", tag="sq", bufs=2)
                    if warm:
                        nc.scalar.square(out=sq, in_=ps)
                    else:
                        nc.vector.tensor_mul(out=sq, in0=qq, in1=qq)
                    ssum = p1.tile([128, 8], F16, name="ssum", tag="ssum",
                                   bufs=4)
                    with nc.allow_low_precision(reason="rms sumsq in f16"):
                        nc.vector.tensor_reduce(
                            ssum, sq.rearrange("p (h d) -> p h d", h=8),
                            axis=mybir.AxisListType.X, op=mybir.AluOpType.add)
                    srt = p1.tile([128, 8], F32, name="srt", tag="srt", bufs=4)
                    nc.scalar.activation(
                        srt, ssum, mybir.ActivationFunctionType.Sqrt,
                        bias=epsc, scale=1.0 / 64.0)
                    rst = p1.tile([128, 8], F16, name="rst", tag="rst", bufs=4)
                    with nc.allow_low_precision(reason="rms recip in f16"):
                        nc.vector.reciprocal(out=rst, in_=srt)
                    # qn = qq * rst (broadcast over d), all-fp16 2x
                    qn = p1.tile([128, 512], F16, name="qn", tag="qn", bufs=2)
                    qn3 = qn.rearrange("p (h d) -> p h d", h=8)
                    rstb = rst.rearrange("p (h o) -> p h o", o=1).to_broadcast(
                        (128, 8, 64))
                    nc.vector.tensor_mul(out=qn3, in0=qq3, in1=rstb)

                    # ---- RoPE in fp16 (DVE 2x): tm = swap(x)*[s|-s]; then
                    # y = x*[c|c] + tm  (2 half-width muls + full mul + add)
                    rot = p1.tile([128, 512], F16, name="rot", tag="rot",
                                  bufs=4)
                    r4 = rot.rearrange("p (h j d) -> p h j d", h=8, j=2)
                    tm = p1.tile([128, 512], F16, name="tm", tag="tm", bufs=2)
                    tm4 = tm.rearrange("p (h j d) -> p h j d", h=8, j=2)
                    c4 = cos_sb[:, tg, :].rearrange(
                        "p (o u i) -> p o u i", o=1, u=1).to_broadcast(
                        (128, 8, 2, 32))
                    s4 = sin_sb[:, tg, :].rearrange(
                        "p (h j d) -> p h j d", h=1, j=2).to_broadcast(
                        (128, 8, 2, 32))
                    qn4 = qn.rearrange("p (h j d) -> p h j d", h=8, j=2)
                    nc.vector.tensor_mul(out=tm4[:, :, 0, :],
                                         in0=qn4[:, :, 1, :],
                                         in1=s4[:, :, 0, :])
                    nc.vector.tensor_mul(out=tm4[:, :, 1, :],
                                         in0=qn4[:, :, 0, :],
                                         in1=s4[:, :, 1, :])
                    nc.vector.tensor_mul(out=r4, in0=qn4, in1=c4)
                    nc.vector.tensor_add(out=r4, in0=r4, in1=tm4)

                    # ---- transpose to feature-major (one bank, one copy) ----
                    dstT = qTa if which == "q" else kTa
                    trans4(rot, dstT[:, :, tg * 128:(tg + 1) * 128],
                           fast_copy=QK_COPY_DVE)

                def pv_head_parcels(qc, h, ptq, ynorm):
                    """Flipped PV for one head as 5 parcels (4 subs +
                    normalize) so they can interleave with S pairs."""
                    yu_box = []

                    def sub_parcel(sub):
                        if sub == 0:
                            yu_box.append((ringp if YU_RING else yupp).tile(
                                [128, 512], F32, name="yu",
                                tag="ring" if YU_RING else "yu", bufs=YU_BUFS))
                        yu3 = yu_box[0].rearrange(
                            "p (s f) -> p s f", s=4)[:, :, 0:65]
                        last_kt = 4 * qc + sub
                        for kt in range(last_kt + 1):
                            nc.tensor.matmul(
                                yu3[:, sub, :],
                                ptq[kt // 2][:, kt % 2,
                                             sub * 128:(sub + 1) * 128],
                                vsb[kt][:, h, :],
                                start=(sub == 0 and kt == 0),
                                stop=(sub == 3 and kt == last_kt),
                                skip_group_check=True)

                    def norm_parcel():
                        yu3 = yu_box[0].rearrange(
                            "p (s f) -> p s f", s=4)[:, :, 0:65]
                        rec = p2.tile([128, 4, 1], F32, name="rec", tag="rec",
                                      bufs=2)
                        nc.vector.reciprocal(out=rec, in_=yu3[:, :, 64:65])
                        nc.vector.tensor_mul(
                            out=ynorm[:, :, h, :], in0=yu3[:, :, 0:64],
                            in1=rec.to_broadcast((128, 4, 64)))

                    return [lambda s=s: sub_parcel(s) for s in range(4)] + \
                        [norm_parcel]

                def do_p2(qc, units, pre_pv):
                    """Attention for query chunk qc; `units` are next-chunk
                    phase-1 closures woven between heads; the first `pre_pv`
                    units are emitted before head 0's PV (qc==0 needs chunk-0
                    v tiles there).  PV runs one head behind S/exp so the PE
                    never waits on the current head's exp."""
                    npair = 2 * qc + 2
                    it = iter(units)
                    ynorm = p2.tile([128, 4, 8, 64], F16, name="yn",
                                    tag="yn", bufs=2)
                    prev = None
                    for h in range(8):
                        hp, b0 = h // 2, 64 * (h % 2)
                        # fillers: previous head's PV parcels + next-chunk
                        # phase-1 units, spread between this head's S pairs
                        fillers = []
                        if prev is not None:
                            fillers.extend(
                                pv_head_parcels(qc, prev[0], prev[1], ynorm))
                        for _ in range(pre_pv if h == 0 else 2):
                            u = next(it, None)
                            if u is not None:
                                fillers.append(u)
                        fidx = [0.0]
                        fstep = len(fillers) / (4 * qc + 4) if SCHED == 2 else 0.0

                        def drain(upto):
                            while fidx[0] < upto and int(fidx[0]) < len(fillers):
                                fillers[int(fidx[0])]()
                                fidx[0] += 1.0

                        if SCHED == 1:
                            drain(len(fillers))
                        ptq = [p2.tile([128, 2, 512], F16, name="ptq",
                                       tag="ptq", bufs=4 * NQ + 2)
                               for _ in range(npair)]
                        for kt in range(4 * qc + 4):
                            m = kt - 4 * qc
                            a0 = max(0, 128 * m)
                            st1 = stpp.tile([128, 512], F32, name="st1",
                                            tag="st1", bufs=ST_BUFS)
                            nc.tensor.matmul(
                                st1[:, a0:512],
                                kTa[b0:b0 + 64, hp,
                                    kt * 128:(kt + 1) * 128],
                                qTa[b0:b0 + 64, hp,
                                    qc * 512 + a0:(qc + 1) * 512],
                                start=True, stop=True)
                            ptt = ptq[kt // 2][:, kt % 2, :]
                            if m < 0 and qc >= 2 and kt < (2 * SCH_Q2 if qc == 2 else 2 * SCH_Q3):
                                # fast exp: f16 bits = int16(A*s + B)
                                nc.vector.tensor_scalar(
                                    out=ptt.bitcast(I16), in0=st1,
                                    scalar1=SCH_A, scalar2=SCH_B,
                                    op0=mybir.AluOpType.mult,
                                    op1=mybir.AluOpType.add)
                            elif m < 0:
                                nc.scalar.activation(
                                    ptt, st1,
                                    mybir.ActivationFunctionType.Exp,
                                    scale=SCALE)
                            else:
                                w0 = 128 * m
                                nc.scalar.activation(
                                    ptt[:, w0:512], st1[:, w0:512],
                                    mybir.ActivationFunctionType.Exp,
                                    scale=SCALE)
                                nc.gpsimd.tensor_mul(
                                    out=ptt[:, w0:w0 + 128],
                                    in0=ptt[:, w0:w0 + 128],
                                    in1=tri16)
                            if SCHED == 2:
                                drain((kt + 1) * fstep)

                        drain(len(fillers))
                        prev = (h, ptq)

                    for f in pv_head_parcels(qc, prev[0], prev[1], ynorm):
                        f()

                    # ---- y -> feature-major fp8; projection; store ----
                    yT8 = p2.tile([128, 4, 512], F16, name="yT8", tag="yT8",
                                  bufs=2)
                    for sub in range(4):
                        u = next(it, None)
                        if u is not None:
                            u()
                        yflat = ynorm[:, sub, :, :].rearrange(
                            "p h d -> p (h d)")
                        trans4(yflat, yT8[:, :, sub * 128:(sub + 1) * 128],
                               fast_copy=True)
                        for jc in range(2):
                            pr = ringp.tile([128, 512], F32, name="pr",
                                            tag="ring", bufs=2)
                            for i in range(4):
                                nc.tensor.matmul(
                                    pr,
                                    yT8[:, i, sub * 128:(sub + 1) * 128],
                                    wp_sb[:, i, jc * 512:(jc + 1) * 512],
                                    start=(i == 0), stop=(i == 3))
                            osb = p2.tile([128, 512], F16, name="osb",
                                          tag="osb", bufs=3)
                            if OSB_DVE and jc == 0:
                                nc.vector.tensor_copy(out=osb, in_=pr)
                            else:
                                nc.scalar.copy(out=osb, in_=pr)
                            r0 = qc * 512 + sub * 128
                            nc.sync.dma_start(
                                out=out_d[r0:r0 + 128,
                                          jc * 512:(jc + 1) * 512],
                                in_=osb)
                    for u in it:
                        u()

                for ii in range(NQ):
                    units = p1_units(ii)
                    if ii == 0:
                        units[0]()                 # x chunk-0 dma first
                        dma_weights_early()        # wq + rope tables
                        units[1]()                 # q ts0
                        dma_weights_late()         # wk, wv, wp
                        for u in units[2:9]:       # q*3 + k*4
                            u()
                        carry = units[9:]          # v*4
                    else:
                        do_p2(ii - 1, carry + units, pre_pv=len(carry))
                        carry = []
                do_p2(NQ - 1, [], pre_pv=0)

    _legalize_waits(nc)
    return nc


def _host_tables():
    inv_freq = 1.0 / (10000.0 ** (np.arange(0, D, 2, dtype=np.float32) / D))
    t = np.arange(T, dtype=np.float32)
    freqs = np.outer(t, inv_freq).astype(np.float32)      # (T, 32)
    cos16 = np.cos(freqs).astype(np.float16)
    s = np.sin(freqs)
    sin16 = np.concatenate([s, -s], axis=1).astype(np.float16)  # [T, 64]
    p = np.arange(128)[:, None]
    f = np.arange(128)[None, :]
    tri = (p <= f).astype(np.float16)                     # (128, 128)
    return cos16, sin16, tri


_CACHE = {}


def kernel(x, v1, wq, wk, wv, wproj, lamb):
    x = np.asarray(x, dtype=np.float32)
    v1 = np.asarray(v1, dtype=np.float32)
    wq = np.asarray(wq, dtype=np.float32)
    wk = np.asarray(wk, dtype=np.float32)
    wv = np.asarray(wv, dtype=np.float32)
    wproj = np.asarray(wproj, dtype=np.float32)
    lam = float(np.asarray(lamb))

    cosn, sinn, tri = _host_tables()

    key = lam
    if key not in _CACHE:
        _CACHE[key] = _build(lam)
    nc = _CACHE[key]

    in_maps = []
    for core in range(8):
        b, hg = core // 2, core % 2
        sl = slice(hg * HG, (hg + 1) * HG)
        in_maps.append({
            "xbT": np.ascontiguousarray(x[b].T).astype(np.float16),
            "v1b": np.ascontiguousarray(lam * v1[b][:, sl]).astype(np.float16),
            "wqT": np.ascontiguousarray(wq[sl, :].T).astype(np.float16),
            "wkT": np.ascontiguousarray(wk[sl, :].T).astype(np.float16),
            "wvT": np.ascontiguousarray(wv[sl, :].T).astype(np.float16),
            "wpT": np.ascontiguousarray(wproj[:, sl].T).astype(np.float16),
            "cosn": cosn,
            "sinn": sinn,
            "tri01": tri,
        })

    res = bass_utils.run_bass_kernel_spmd(nc, in_maps, core_ids=list(range(8)))
    y = np.empty((B, T, C), dtype=np.float32)
    for b in range(B):
        y[b] = (res.results[2 * b]["out"].astype(np.float32)
                + res.results[2 * b + 1]["out"].astype(np.float32))
    return (y, v1)


# revision 4
# speedup vs baseline: 1.0148x; 1.0062x over previous
"""Causal self-attention (B=4, T=2048, C=1024, H=16, D=64) on 8 TRN2 cores, v2.

Sharding: core = 2*b + hg  (b = batch 0..3, hg = head-group 0..1 of 8 heads).
Host passes x^T and the wq/wk/wv/wproj slices transposed; weights pre-scaled
x64 so fp8e4 uses its normal range (1/64 folded into the v-blend scalar and
the output scale; q/k need no correction since RMS norm is scale invariant).

Per-core pipeline:
  p1 (per 128-token tile): QKV via fp8 DoubleRow matmuls; RMS stats + scale
     into fp16; RoPE in fp16 on DVE (2x mode); 4 PE transposes batched into
     one PSUM bank (shared zero-region start/stop) + one wide DVE copy into
     feature-major qT/kT; v blended into [128,8,65] fp16 tiles whose col 64
     is 1.0.
  p2 (per 512-query chunk, per head): S^T = kT.T @ qT fp16 into PSUM pairs
     [128,2,512]; exp on ACT batched per pair (diagonal pairs windowed + fp16
     triangle multiply); flipped PV (out[128 tokens, 65] per head, pt
     stationary, v moving) accumulating softmax denominators via the ones
     column; gpsimd normalize_recip divides per partition; y transposed back
     to feature-major fp8 (batched-bank + one copy); fp8 DoubleRow output
     projection; fp16 store.

The ISA has ONE semaphore-wait slot per instruction; Tile emits more.
_legalize_waits() splits extras onto same-engine NoOps post-scheduling.
"""

import math

import numpy as np
import ml_dtypes

import concourse.bass as bass
import concourse.mybir as mybir
import concourse.tile as tile
from concourse import bass_utils
from concourse.masks import make_identity

F32 = mybir.dt.float32
F16 = mybir.dt.float16
I16 = mybir.dt.int16
F8 = mybir.dt.float8e4
NPF8 = ml_dtypes.float8_e4m3
DR = mybir.MatmulPerfMode.DoubleRow

B, T, C, H, D = 4, 2048, 1024, 16, 64
HG = C // 2          # 512 features per head group (8 heads x 64)
NT = T // 128        # 16 t-tiles
NQ = T // 512        # 4 query chunks
EPS = 1.1920928955078125e-07
SCALE = 1.0 / math.sqrt(D)  # 0.125
WSCALE = 1.0         # no weight prescale needed at fp16
# Schraudolph fast-exp (f16 bit trick) for the first N below-diagonal pairs
# of every (chunk, head): exp(SCALE*s) ~= bitcast_f16(int16(A*s + B))
N_SCHRAUDOLPH = 2
SCHED = 0
WARM_TILES = 6
OSB_DVE = True
QK_COPY_DVE = False
SCH_Q1 = 1
SCH_Q2 = 2
SCH_Q3 = 4
ST_BUFS = 5
YU_RING = True
YU_BUFS = 2
SCH_A = 1024.0 / math.log(2.0) * SCALE
SCH_B = 15360.0 - 59.27

_wsplit_counter = [0]


def _legalize_waits(nc):
    """Split multi-wait instructions into single-wait NoOp chains."""
    n = 0
    for f in nc.m.functions:
        for bb in f.blocks:
            new_list = []
            changed = False
            for inst in bb.instructions:
                si = inst.sync_info
                if si is not None and si.on_wait and len(si.on_wait) > 1:
                    waits = list(si.on_wait)
                    for w in waits[:-1]:
                        _wsplit_counter[0] += 1
                        new_list.append(mybir.InstNoOp(
                            name=f"WSPLIT-{_wsplit_counter[0]}",
                            engine=inst.engine, ins=[], outs=[],
                            sync_info=mybir.SyncInfo(on_wait=[w], on_update=[]),
                        ))
                    si.on_wait = waits[-1:]
                    changed = True
                    n += 1
                new_list.append(inst)
            if changed:
                bb.instructions = new_list
    return n


def _build(lam: float) -> bass.Bass:
    nc = bass.Bass("TRN2", target_bir_lowering=False, debug=False, num_devices=8)

    xb_d = nc.dram_tensor("xbT", [C, T], F16, kind="ExternalInput").ap()
    v1_d = nc.dram_tensor("v1b", [T, HG], F16, kind="ExternalInput").ap()
    wq_d = nc.dram_tensor("wqT", [C, HG], F16, kind="ExternalInput").ap()
    wk_d = nc.dram_tensor("wkT", [C, HG], F16, kind="ExternalInput").ap()
    wv_d = nc.dram_tensor("wvT", [C, HG], F16, kind="ExternalInput").ap()
    wp_d = nc.dram_tensor("wpT", [HG, C], F16, kind="ExternalInput").ap()
    cos_d = nc.dram_tensor("cosn", [T, 32], F16, kind="ExternalInput").ap()
    sin_d = nc.dram_tensor("sinn", [T, 64], F16, kind="ExternalInput").ap()
    tri_d = nc.dram_tensor("tri01", [128, 128], F16, kind="ExternalInput").ap()
    out_d = nc.dram_tensor("out", [T, C], F16, kind="ExternalOutput").ap()

    with tile.TileContext(nc) as tc:
        with (
            tc.tile_pool(name="const", bufs=1) as const,
            tc.tile_pool(name="pers", bufs=1) as pers,
        ):
            identf = const.tile([128, 128], F32)
            make_identity(nc, identf)
            ident = const.tile([128, 128], F16)
            nc.scalar.copy(out=ident, in_=identf)
            tri16 = const.tile([128, 128], F16)
            nc.gpsimd.dma_start(out=tri16, in_=tri_d)
            epsc = const.tile([128, 1], F32)
            nc.vector.memset(epsc, EPS)

            # persistent feature-major q/k (fp16; dim1 = head-pair j) and v
            # tiles (fp16, col 64 = 1.0 accumulates softmax denominators)
            qTa = pers.tile([128, 4, T], F16, name="qTa", tag="qTa")
            kTa = pers.tile([128, 4, T], F16, name="kTa", tag="kTa")
            vsb = [pers.tile([128, 8, 65], F16, name=f"v{t}", tag=f"v{t}")
                   for t in range(NT)]
            for t in range(NT):
                nc.gpsimd.memset(vsb[t][:, :, 64:65], 1.0)

            with (
                tc.tile_pool(name="p1", bufs=1) as p1,
                tc.tile_pool(name="ring", bufs=1, space="PSUM") as ringp,
                tc.tile_pool(name="stp", bufs=1, space="PSUM") as stpp,
                tc.tile_pool(name="yup", bufs=1, space="PSUM") as yupp,
                tc.tile_pool(name="tpp", bufs=1, space="PSUM") as tppp,
                tc.tile_pool(name="p2", bufs=1) as p2,
            ):
                wq_sb = p1.tile([128, 8, HG], F16)
                wk_sb = p1.tile([128, 8, HG], F16)
                wv_sb = p1.tile([128, 8, HG], F16)
                wp_sb = p1.tile([128, 4, C], F16)
                cos_sb = p1.tile([128, NT, 32], F16)
                sin_sb = p1.tile([128, NT, 64], F16)

                def dma_weights_early():
                    nc.sync.dma_start(
                        out=wq_sb,
                        in_=wq_d.rearrange("(c p) i -> p c i", p=128))
                    nc.sync.dma_start(
                        out=cos_sb,
                        in_=cos_d.rearrange("(n p) i -> p n i", p=128))
                    nc.sync.dma_start(
                        out=sin_sb,
                        in_=sin_d.rearrange("(n p) i -> p n i", p=128))

                def dma_weights_late():
                    nc.sync.dma_start(
                        out=wk_sb,
                        in_=wk_d.rearrange("(c p) i -> p c i", p=128))
                    nc.sync.dma_start(
                        out=wv_sb,
                        in_=wv_d.rearrange("(c p) i -> p c i", p=128))
                    nc.sync.dma_start(
                        out=wp_sb,
                        in_=wp_d.rearrange("(c p) j -> p c j", p=128))

                w_sb = {"q": wq_sb, "k": wk_sb, "v": wv_sb}

                def trans4(src, dst_ap, fast_copy):
                    """4 batched [128,128] transposes of fp16 `src` through one
                    PSUM bank, then a single wide copy to dst_ap ([128,4,128]).
                    """
                    tp = tppp.tile([128, 4, 128], F16, name="tp", tag="tp",
                                   bufs=1)
                    for j in range(4):
                        nc.tensor.matmul(
                            tp[:, j, :], src[:, j * 128:(j + 1) * 128], ident,
                            start=(j == 0), stop=(j == 3), is_transpose=True,
                            skip_group_check=True)
                    if fast_copy:
                        nc.vector.tensor_copy(out=dst_ap, in_=tp)
                    else:
                        nc.scalar.copy(out=dst_ap, in_=tp)

                def p1_units(tc4):
                    """Phase-1 closures for chunk tc4: dma, q*4, k*4, v*4."""
                    xT = p1.tile([128, 8, 512], F16, name="xT", tag="xT",
                                 bufs=2)
                    t0 = tc4 * 512

                    def dma_x():
                        nc.sync.dma_start(
                            out=xT,
                            in_=xb_d[:, t0:t0 + 512].rearrange(
                                "(c p) t -> p c t", p=128))
                    units = [dma_x]
                    for which in ("q", "k", "v"):
                        for ts in range(4):
                            tg = tc4 * 4 + ts
                            units.append(lambda ts=ts, tg=tg, which=which: (
                                p1_unit(xT, ts, tg, which)))
                    return units

                def p1_unit(xT, ts, tg, which):
                    ps = ringp.tile([128, 512], F32, name="ps", tag="ring",
                                    bufs=2)
                    for cc in range(8):
                        nc.tensor.matmul(
                            ps,
                            xT[:, cc, ts * 128:(ts + 1) * 128],
                            w_sb[which][:, cc, :],
                            start=(cc == 0), stop=(cc == 7))
                    p3 = ps.rearrange("p (h d) -> p h d", h=8)
                    if which == "v":
                        v1t = p1.tile([128, HG], F16, name="v1t",
                                      tag="v1t", bufs=2)
                        nc.sync.dma_start(
                            out=v1t, in_=v1_d[tg * 128:(tg + 1) * 128, :])
                        # v = (1-lam)/64 * (x@wv') + lam*v1 (v1 pre-scaled)
                        nc.vector.scalar_tensor_tensor(
                            out=vsb[tg][:, :, 0:64],
                            in0=p3,
                            scalar=(1.0 - lam),
                            in1=v1t.rearrange("p (h d) -> p h d", h=8),
                            op0=mybir.AluOpType.mult,
                            op1=mybir.AluOpType.add)
                        return

                    # ---- copy psum to fp16 once (frees the PSUM ring
                    # fast; everything downstream runs in DVE 2x mode) ----
                    warm = tg < WARM_TILES  # chunk 0: ACT is idle, help out
                    qq = p1.tile([128, 512], F16, name="qq", tag="qq", bufs=3)
                    qq3 = qq.rearrange("p (h d) -> p h d", h=8)
                    if warm:
                        nc.scalar.copy(out=qq, in_=ps)
                    else:
                        nc.vector.tensor_copy(out=qq, in_=ps)
                    # ---- RMS stats (q/k) ----
                    sq = p1.tile([128, 512], F16, name="sq", tag="sq", bufs=2)
                    if warm:
                        nc.scalar.square(out=sq, in_=ps)
                    else:
                        nc.vector.tensor_mul(out=sq, in0=qq, in1=qq)
                    ssum = p1.tile([128, 8], F16, name="ssum", tag="ssum",
                                   bufs=4)
                    with nc.allow_low_precision(reason="rms sumsq in f16"):
                        nc.vector.tensor_reduce(
                            ssum, sq.rearrange("p (h d) -> p h d", h=8),
                            axis=mybir.AxisListType.X, op=mybir.AluOpType.add)
                    srt = p1.tile([128, 8], F32, name="srt", tag="srt", bufs=4)
                    nc.scalar.activation(
                        srt, ssum, mybir.ActivationFunctionType.Sqrt,
                        bias=epsc, scale=1.0 / 64.0)
                    rst = p1.tile([128, 8], F16, name="rst", tag="rst", bufs=4)
                    with nc.allow_low_precision(reason="rms recip in f16"):
                        nc.vector.reciprocal(out=rst, in_=srt)
                    # qn = qq * rst (broadcast over d), all-fp16 2x
                    qn = p1.tile([128, 512], F16, name="qn", tag="qn", bufs=2)
                    qn3 = qn.rearrange("p (h d) -> p h d", h=8)
                    rstb = rst.rearrange("p (h o) -> p h o", o=1).to_broadcast(
                        (128, 8, 64))
                    nc.vector.tensor_mul(out=qn3, in0=qq3, in1=rstb)

                    # ---- RoPE in fp16 (DVE 2x): tm = swap(x)*[s|-s]; then
                    # y = x*[c|c] + tm  (2 half-width muls + full mul + add)
                    rot = p1.tile([128, 512], F16, name="rot", tag="rot",
                                  bufs=4)
                    r4 = rot.rearrange("p (h j d) -> p h j d", h=8, j=2)
                    tm = p1.tile([128, 512], F16, name="tm", tag="tm", bufs=2)
                    tm4 = tm.rearrange("p (h j d) -> p h j d", h=8, j=2)
                    c4 = cos_sb[:, tg, :].rearrange(
                        "p (o u i) -> p o u i", o=1, u=1).to_broadcast(
                        (128, 8, 2, 32))
                    s4 = sin_sb[:, tg, :].rearrange(
                        "p (h j d) -> p h j d", h=1, j=2).to_broadcast(
                        (128, 8, 2, 32))
                    qn4 = qn.rearrange("p (h j d) -> p h j d", h=8, j=2)
                    nc.vector.tensor_mul(out=tm4[:, :, 0, :],
                                         in0=qn4[:, :, 1, :],
                                         in1=s4[:, :, 0, :])
                    nc.vector.tensor_mul(out=tm4[:, :, 1, :],
                                         in0=qn4[:, :, 0, :],
                                         in1=s4[:, :, 1, :])
                    nc.vector.tensor_mul(out=r4, in0=qn4, in1=c4)
                    nc.vector.tensor_add(out=r4, in0=r4, in1=tm4)

                    # ---- transpose to feature-major (one bank, one copy) ----
                    dstT = qTa if which == "q" else kTa
                    trans4(rot, dstT[:, :, tg * 128:(tg + 1) * 128],
                           fast_copy=QK_COPY_DVE)

                def pv_head_parcels(qc, h, ptq, ynorm):
                    """Flipped PV for one head as 5 parcels (4 subs +
                    normalize) so they can interleave with S pairs."""
                    yu_box = []

                    def sub_parcel(sub):
                        if sub == 0:
                            yu_box.append((ringp if YU_RING else yupp).tile(
                                [128, 512], F32, name="yu",
                                tag="ring" if YU_RING else "yu", bufs=YU_BUFS))
                        yu3 = yu_box[0].rearrange(
                            "p (s f) -> p s f", s=4)[:, :, 0:65]
                        last_kt = 4 * qc + sub
                        for kt in range(last_kt + 1):
                            nc.tensor.matmul(
                                yu3[:, sub, :],
                                ptq[kt // 2][:, kt % 2,
                                             sub * 128:(sub + 1) * 128],
                                vsb[kt][:, h, :],
                                start=(sub == 0 and kt == 0),
                                stop=(sub == 3 and kt == last_kt),
                                skip_group_check=True)

                    def norm_parcel():
                        yu3 = yu_box[0].rearrange(
                            "p (s f) -> p s f", s=4)[:, :, 0:65]
                        rec = p2.tile([128, 4, 1], F32, name="rec", tag="rec",
                                      bufs=2)
                        nc.vector.reciprocal(out=rec, in_=yu3[:, :, 64:65])
                        nc.vector.tensor_mul(
                            out=ynorm[:, :, h, :], in0=yu3[:, :, 0:64],
                            in1=rec.to_broadcast((128, 4, 64)))

                    return [lambda s=s: sub_parcel(s) for s in range(4)] + \
                        [norm_parcel]

                def do_p2(qc, units, pre_pv):
                    """Attention for query chunk qc; `units` are next-chunk
                    phase-1 closures woven between heads; the first `pre_pv`
                    units are emitted before head 0's PV (qc==0 needs chunk-0
                    v tiles there).  PV runs one head behind S/exp so the PE
                    never waits on the current head's exp."""
                    npair = 2 * qc + 2
                    it = iter(units)
                    ynorm = p2.tile([128, 4, 8, 64], F16, name="yn",
                                    tag="yn", bufs=2)
                    prev = None
                    for h in range(8):
                        hp, b0 = h // 2, 64 * (h % 2)
                        # fillers: previous head's PV parcels + next-chunk
                        # phase-1 units, spread between this head's S pairs
                        fillers = []
                        if prev is not None:
                            fillers.extend(
                                pv_head_parcels(qc, prev[0], prev[1], ynorm))
                        for _ in range(pre_pv if h == 0 else 2):
                            u = next(it, None)
                            if u is not None:
                                fillers.append(u)
                        fidx = [0.0]
                        fstep = len(fillers) / (4 * qc + 4) if SCHED == 2 else 0.0

                        def drain(upto):
                            while fidx[0] < upto and int(fidx[0]) < len(fillers):
                                fillers[int(fidx[0])]()
                                fidx[0] += 1.0

                        if SCHED == 1:
                            drain(len(fillers))
                        ptq = [p2.tile([128, 2, 512], F16, name="ptq",
                                       tag="ptq", bufs=4 * NQ + 2)
                               for _ in range(npair)]
                        for kt in range(4 * qc + 4):
                            m = kt - 4 * qc
                            a0 = max(0, 128 * m)
                            st1 = stpp.tile([128, 512], F32, name="st1",
                                            tag="st1", bufs=ST_BUFS)
                            nc.tensor.matmul(
                                st1[:, a0:512],
                                kTa[b0:b0 + 64, hp,
                                    kt * 128:(kt + 1) * 128],
                                qTa[b0:b0 + 64, hp,
                                    qc * 512 + a0:(qc + 1) * 512],
                                start=True, stop=True)
                            ptt = ptq[kt // 2][:, kt % 2, :]
                            if m < 0 and kt < {1: 2 * SCH_Q1, 2: 2 * SCH_Q2, 3: 2 * SCH_Q3}.get(qc, 0):
                                # fast exp: f16 bits = int16(A*s + B)
                                nc.vector.tensor_scalar(
                                    out=ptt.bitcast(I16), in0=st1,
                                    scalar1=SCH_A, scalar2=SCH_B,
                                    op0=mybir.AluOpType.mult,
                                    op1=mybir.AluOpType.add)
                            elif m < 0:
                                nc.scalar.activation(
                                    ptt, st1,
                                    mybir.ActivationFunctionType.Exp,
                                    scale=SCALE)
                            else:
                                w0 = 128 * m
                                nc.scalar.activation(
                                    ptt[:, w0:512], st1[:, w0:512],
                                    mybir.ActivationFunctionType.Exp,
                                    scale=SCALE)
                                nc.gpsimd.tensor_mul(
                                    out=ptt[:, w0:w0 + 128],
                                    in0=ptt[:, w0:w0 + 128],
                                    in1=tri16)
                            if SCHED == 2:
                                drain((kt + 1) * fstep)

                        drain(len(fillers))
                        prev = (h, ptq)

                    for f in pv_head_parcels(qc, prev[0], prev[1], ynorm):
                        f()

                    # ---- y -> feature-major fp8; projection; store ----
                    yT8 = p2.tile([128, 4, 512], F16, name="yT8", tag="yT8",
                                  bufs=2)
                    for sub in range(4):
                        u = next(it, None)
                        if u is not None:
                            u()
                        yflat = ynorm[:, sub, :, :].rearrange(
                            "p h d -> p (h d)")
                        trans4(yflat, yT8[:, :, sub * 128:(sub + 1) * 128],
                               fast_copy=True)
                        for jc in range(2):
                            pr = ringp.tile([128, 512], F32, name="pr",
                                            tag="ring", bufs=2)
                            for i in range(4):
                                nc.tensor.matmul(
                                    pr,
                                    yT8[:, i, sub * 128:(sub + 1) * 128],
                                    wp_sb[:, i, jc * 512:(jc + 1) * 512],
                                    start=(i == 0), stop=(i == 3))
                            osb = p2.tile([128, 512], F16, name="osb",
                                          tag="osb", bufs=3)
                            if OSB_DVE and jc == 0:
                                nc.vector.tensor_copy(out=osb, in_=pr)
                            else:
                                nc.scalar.copy(out=osb, in_=pr)
                            r0 = qc * 512 + sub * 128
                            nc.sync.dma_start(
                                out=out_d[r0:r0 + 128,
                                          jc * 512:(jc + 1) * 512],
                                in_=osb)
                    for u in it:
                        u()

                for ii in range(NQ):
                    units = p1_units(ii)
                    if ii == 0:
                        units[0]()                 # x chunk-0 dma first
                        dma_weights_early()        # wq + rope tables
                        units[1]()                 # q ts0
                        dma_weights_late()         # wk, wv, wp
                        for u in units[2:9]:       # q*3 + k*4
                            u()
                        carry = units[9:]          # v*4
                    else:
                        do_p2(ii - 1, carry + units, pre_pv=len(carry))
                        carry = []
                do_p2(NQ - 1, [], pre_pv=0)

    _legalize_waits(nc)
    return nc


def _host_tables():
    inv_freq = 1.0 / (10000.0 ** (np.arange(0, D, 2, dtype=np.float32) / D))
    t = np.arange(T, dtype=np.float32)
    freqs = np.outer(t, inv_freq).astype(np.float32)      # (T, 32)
    cos16 = np.cos(freqs).astype(np.float16)
    s = np.sin(freqs)
    sin16 = np.concatenate([s, -s], axis=1).astype(np.float16)  # [T, 64]
    p = np.arange(128)[:, None]
    f = np.arange(128)[None, :]
    tri = (p <= f).astype(np.float16)                     # (128, 128)
    return cos16, sin16, tri


_CACHE = {}


def kernel(x, v1, wq, wk, wv, wproj, lamb):
    x = np.asarray(x, dtype=np.float32)
    v1 = np.asarray(v1, dtype=np.float32)
    wq = np.asarray(wq, dtype=np.float32)
    wk = np.asarray(wk, dtype=np.float32)
    wv = np.asarray(wv, dtype=np.float32)
    wproj = np.asarray(wproj, dtype=np.float32)
    lam = float(np.asarray(lamb))

    cosn, sinn, tri = _host_tables()

    key = lam
    if key not in _CACHE:
        _CACHE[key] = _build(lam)
    nc = _CACHE[key]

    in_maps = []
    for core in range(8):
        b, hg = core // 2, core % 2
        sl = slice(hg * HG, (hg + 1) * HG)
        in_maps.append({
            "xbT": np.ascontiguousarray(x[b].T).astype(np.float16),
            "v1b": np.ascontiguousarray(lam * v1[b][:, sl]).astype(np.float16),
            "wqT": np.ascontiguousarray(wq[sl, :].T).astype(np.float16),
            "wkT": np.ascontiguousarray(wk[sl, :].T).astype(np.float16),
            "wvT": np.ascontiguousarray(wv[sl, :].T).astype(np.float16),
            "wpT": np.ascontiguousarray(wproj[:, sl].T).astype(np.float16),
            "cosn": cosn,
            "sinn": sinn,
            "tri01": tri,
        })

    res = bass_utils.run_bass_kernel_spmd(nc, in_maps, core_ids=list(range(8)))
    y = np.empty((B, T, C), dtype=np.float32)
    for b in range(B):
        y[b] = (res.results[2 * b]["out"].astype(np.float32)
                + res.results[2 * b + 1]["out"].astype(np.float32))
    return (y, v1)


# revision 5
# speedup vs baseline: 1.0262x; 1.0112x over previous
"""Causal self-attention (B=4, T=2048, C=1024, H=16, D=64) on 8 TRN2 cores, v2.

Sharding: core = 2*b + hg  (b = batch 0..3, hg = head-group 0..1 of 8 heads).
Host passes x^T and the wq/wk/wv/wproj slices transposed; weights pre-scaled
x64 so fp8e4 uses its normal range (1/64 folded into the v-blend scalar and
the output scale; q/k need no correction since RMS norm is scale invariant).

Per-core pipeline:
  p1 (per 128-token tile): QKV via fp8 DoubleRow matmuls; RMS stats + scale
     into fp16; RoPE in fp16 on DVE (2x mode); 4 PE transposes batched into
     one PSUM bank (shared zero-region start/stop) + one wide DVE copy into
     feature-major qT/kT; v blended into [128,8,65] fp16 tiles whose col 64
     is 1.0.
  p2 (per 512-query chunk, per head): S^T = kT.T @ qT fp16 into PSUM pairs
     [128,2,512]; exp on ACT batched per pair (diagonal pairs windowed + fp16
     triangle multiply); flipped PV (out[128 tokens, 65] per head, pt
     stationary, v moving) accumulating softmax denominators via the ones
     column; gpsimd normalize_recip divides per partition; y transposed back
     to feature-major fp8 (batched-bank + one copy); fp8 DoubleRow output
     projection; fp16 store.

The ISA has ONE semaphore-wait slot per instruction; Tile emits more.
_legalize_waits() splits extras onto same-engine NoOps post-scheduling.
"""

import math

import numpy as np
import ml_dtypes

import concourse.bass as bass
import concourse.mybir as mybir
import concourse.tile as tile
from concourse import bass_utils
from concourse.masks import make_identity

F32 = mybir.dt.float32
F16 = mybir.dt.float16
I16 = mybir.dt.int16
F8 = mybir.dt.float8e4
NPF8 = ml_dtypes.float8_e4m3
DR = mybir.MatmulPerfMode.DoubleRow

B, T, C, H, D = 4, 2048, 1024, 16, 64
HG = C // 2          # 512 features per head group (8 heads x 64)
NT = T // 128        # 16 t-tiles
NQ = T // 512        # 4 query chunks
EPS = 1.1920928955078125e-07
SCALE = 1.0 / math.sqrt(D)  # 0.125
WSCALE = 1.0         # no weight prescale needed at fp16
# Schraudolph fast-exp (f16 bit trick) for the first N below-diagonal pairs
# of every (chunk, head): exp(SCALE*s) ~= bitcast_f16(int16(A*s + B))
N_SCHRAUDOLPH = 2
SCHED = 0
WARM_TILES = 6
OSB_DVE = True
QK_COPY_DVE = False
SCH_Q1 = 1
TRI_DVE = False
YTP_DVE = True
V1_SP = True
TP_BUFS = 1
KT_REV = True
SCH_Q2 = 2
SCH_Q3 = 4
ST_BUFS = 5
YU_RING = True
YU_BUFS = 2
SCH_A = 1024.0 / math.log(2.0) * SCALE
SCH_B = 15360.0 - 59.27

_wsplit_counter = [0]


def _legalize_waits(nc):
    """Split multi-wait instructions into single-wait NoOp chains."""
    n = 0
    for f in nc.m.functions:
        for bb in f.blocks:
            new_list = []
            changed = False
            for inst in bb.instructions:
                si = inst.sync_info
                if si is not None and si.on_wait and len(si.on_wait) > 1:
                    waits = list(si.on_wait)
                    for w in waits[:-1]:
                        _wsplit_counter[0] += 1
                        new_list.append(mybir.InstNoOp(
                            name=f"WSPLIT-{_wsplit_counter[0]}",
                            engine=inst.engine, ins=[], outs=[],
                            sync_info=mybir.SyncInfo(on_wait=[w], on_update=[]),
                        ))
                    si.on_wait = waits[-1:]
                    changed = True
                    n += 1
                new_list.append(inst)
            if changed:
                bb.instructions = new_list
    return n


def _build(lam: float) -> bass.Bass:
    nc = bass.Bass("TRN2", target_bir_lowering=False, debug=False, num_devices=8)

    xb_d = nc.dram_tensor("xbT", [C, T], F16, kind="ExternalInput").ap()
    v1_d = nc.dram_tensor("v1b", [T, HG], F16, kind="ExternalInput").ap()
    wq_d = nc.dram_tensor("wqT", [C, HG], F16, kind="ExternalInput").ap()
    wk_d = nc.dram_tensor("wkT", [C, HG], F16, kind="ExternalInput").ap()
    wv_d = nc.dram_tensor("wvT", [C, HG], F16, kind="ExternalInput").ap()
    wp_d = nc.dram_tensor("wpT", [HG, C], F16, kind="ExternalInput").ap()
    cos_d = nc.dram_tensor("cosn", [T, 32], F16, kind="ExternalInput").ap()
    sin_d = nc.dram_tensor("sinn", [T, 64], F16, kind="ExternalInput").ap()
    tri_d = nc.dram_tensor("tri01", [128, 128], F16, kind="ExternalInput").ap()
    out_d = nc.dram_tensor("out", [T, C], F16, kind="ExternalOutput").ap()

    with tile.TileContext(nc) as tc:
        with (
            tc.tile_pool(name="const", bufs=1) as const,
            tc.tile_pool(name="pers", bufs=1) as pers,
        ):
            identf = const.tile([128, 128], F32)
            make_identity(nc, identf)
            ident = const.tile([128, 128], F16)
            nc.scalar.copy(out=ident, in_=identf)
            tri16 = const.tile([128, 128], F16)
            nc.gpsimd.dma_start(out=tri16, in_=tri_d)
            epsc = const.tile([128, 1], F32)
            nc.vector.memset(epsc, EPS)

            # persistent feature-major q/k (fp16; dim1 = head-pair j) and v
            # tiles (fp16, col 64 = 1.0 accumulates softmax denominators)
            qTa = pers.tile([128, 4, T], F16, name="qTa", tag="qTa")
            kTa = pers.tile([128, 4, T], F16, name="kTa", tag="kTa")
            vsb = [pers.tile([128, 8, 65], F16, name=f"v{t}", tag=f"v{t}")
                   for t in range(NT)]
            for t in range(NT):
                nc.gpsimd.memset(vsb[t][:, :, 64:65], 1.0)

            with (
                tc.tile_pool(name="p1", bufs=1) as p1,
                tc.tile_pool(name="ring", bufs=1, space="PSUM") as ringp,
                tc.tile_pool(name="stp", bufs=1, space="PSUM") as stpp,
                tc.tile_pool(name="yup", bufs=1, space="PSUM") as yupp,
                tc.tile_pool(name="tpp", bufs=1, space="PSUM") as tppp,
                tc.tile_pool(name="p2", bufs=1) as p2,
            ):
                wq_sb = p1.tile([128, 8, HG], F16)
                wk_sb = p1.tile([128, 8, HG], F16)
                wv_sb = p1.tile([128, 8, HG], F16)
                wp_sb = p1.tile([128, 4, C], F16)
                cos_sb = p1.tile([128, NT, 32], F16)
                sin_sb = p1.tile([128, NT, 64], F16)

                def dma_weights_early():
                    nc.sync.dma_start(
                        out=wq_sb,
                        in_=wq_d.rearrange("(c p) i -> p c i", p=128))
                    nc.sync.dma_start(
                        out=cos_sb,
                        in_=cos_d.rearrange("(n p) i -> p n i", p=128))
                    nc.sync.dma_start(
                        out=sin_sb,
                        in_=sin_d.rearrange("(n p) i -> p n i", p=128))

                def dma_weights_late():
                    nc.sync.dma_start(
                        out=wk_sb,
                        in_=wk_d.rearrange("(c p) i -> p c i", p=128))
                    nc.sync.dma_start(
                        out=wv_sb,
                        in_=wv_d.rearrange("(c p) i -> p c i", p=128))
                    nc.sync.dma_start(
                        out=wp_sb,
                        in_=wp_d.rearrange("(c p) j -> p c j", p=128))

                w_sb = {"q": wq_sb, "k": wk_sb, "v": wv_sb}

                def trans4(src, dst_ap, fast_copy):
                    """4 batched [128,128] transposes of fp16 `src` through one
                    PSUM bank, then a single wide copy to dst_ap ([128,4,128]).
                    """
                    tp = tppp.tile([128, 4, 128], F16, name="tp", tag="tp",
                                   bufs=TP_BUFS)
                    for j in range(4):
                        nc.tensor.matmul(
                            tp[:, j, :], src[:, j * 128:(j + 1) * 128], ident,
                            start=(j == 0), stop=(j == 3), is_transpose=True,
                            skip_group_check=True)
                    if fast_copy:
                        nc.vector.tensor_copy(out=dst_ap, in_=tp)
                    else:
                        nc.scalar.copy(out=dst_ap, in_=tp)

                def p1_units(tc4):
                    """Phase-1 closures for chunk tc4: dma, q*4, k*4, v*4."""
                    xT = p1.tile([128, 8, 512], F16, name="xT", tag="xT",
                                 bufs=2)
                    t0 = tc4 * 512

                    def dma_x():
                        nc.sync.dma_start(
                            out=xT,
                            in_=xb_d[:, t0:t0 + 512].rearrange(
                                "(c p) t -> p c t", p=128))
                    units = [dma_x]
                    for which in ("q", "k", "v"):
                        for ts in range(4):
                            tg = tc4 * 4 + ts
                            units.append(lambda ts=ts, tg=tg, which=which: (
                                p1_unit(xT, ts, tg, which)))
                    return units

                def p1_unit(xT, ts, tg, which):
                    ps = ringp.tile([128, 512], F32, name="ps", tag="ring",
                                    bufs=2)
                    for cc in range(8):
                        nc.tensor.matmul(
                            ps,
                            xT[:, cc, ts * 128:(ts + 1) * 128],
                            w_sb[which][:, cc, :],
                            start=(cc == 0), stop=(cc == 7))
                    p3 = ps.rearrange("p (h d) -> p h d", h=8)
                    if which == "v":
                        v1t = p1.tile([128, HG], F16, name="v1t",
                                      tag="v1t", bufs=2)
                        (nc.sync if V1_SP else nc.scalar).dma_start(
                            out=v1t, in_=v1_d[tg * 128:(tg + 1) * 128, :])
                        # v = (1-lam)/64 * (x@wv') + lam*v1 (v1 pre-scaled)
                        nc.vector.scalar_tensor_tensor(
                            out=vsb[tg][:, :, 0:64],
                            in0=p3,
                            scalar=(1.0 - lam),
                            in1=v1t.rearrange("p (h d) -> p h d", h=8),
                            op0=mybir.AluOpType.mult,
                            op1=mybir.AluOpType.add)
                        return

                    # ---- copy psum to fp16 once (frees the PSUM ring
                    # fast; everything downstream runs in DVE 2x mode) ----
                    warm = tg < WARM_TILES  # chunk 0: ACT is idle, help out
                    qq = p1.tile([128, 512], F16, name="qq", tag="qq", bufs=3)
                    qq3 = qq.rearrange("p (h d) -> p h d", h=8)
                    if warm:
                        nc.scalar.copy(out=qq, in_=ps)
                    else:
                        nc.vector.tensor_copy(out=qq, in_=ps)
                    # ---- RMS stats (q/k) ----
                    sq = p1.tile([128, 512], F16, name="sq", tag="sq", bufs=2)
                    if warm:
                        nc.scalar.square(out=sq, in_=ps)
                    else:
                        nc.vector.tensor_mul(out=sq, in0=qq, in1=qq)
                    ssum = p1.tile([128, 8], F16, name="ssum", tag="ssum",
                                   bufs=4)
                    with nc.allow_low_precision(reason="rms sumsq in f16"):
                        nc.vector.tensor_reduce(
                            ssum, sq.rearrange("p (h d) -> p h d", h=8),
                            axis=mybir.AxisListType.X, op=mybir.AluOpType.add)
                    srt = p1.tile([128, 8], F32, name="srt", tag="srt", bufs=4)
                    nc.scalar.activation(
                        srt, ssum, mybir.ActivationFunctionType.Sqrt,
                        bias=epsc, scale=1.0 / 64.0)
                    rst = p1.tile([128, 8], F16, name="rst", tag="rst", bufs=4)
                    with nc.allow_low_precision(reason="rms recip in f16"):
                        nc.vector.reciprocal(out=rst, in_=srt)
                    # qn = qq * rst (broadcast over d), all-fp16 2x
                    qn = p1.tile([128, 512], F16, name="qn", tag="qn", bufs=2)
                    qn3 = qn.rearrange("p (h d) -> p h d", h=8)
                    rstb = rst.rearrange("p (h o) -> p h o", o=1).to_broadcast(
                        (128, 8, 64))
                    nc.vector.tensor_mul(out=qn3, in0=qq3, in1=rstb)

                    # ---- RoPE in fp16 (DVE 2x): tm = swap(x)*[s|-s]; then
                    # y = x*[c|c] + tm  (2 half-width muls + full mul + add)
                    rot = p1.tile([128, 512], F16, name="rot", tag="rot",
                                  bufs=4)
                    r4 = rot.rearrange("p (h j d) -> p h j d", h=8, j=2)
                    tm = p1.tile([128, 512], F16, name="tm", tag="tm", bufs=2)
                    tm4 = tm.rearrange("p (h j d) -> p h j d", h=8, j=2)
                    c4 = cos_sb[:, tg, :].rearrange(
                        "p (o u i) -> p o u i", o=1, u=1).to_broadcast(
                        (128, 8, 2, 32))
                    s4 = sin_sb[:, tg, :].rearrange(
                        "p (h j d) -> p h j d", h=1, j=2).to_broadcast(
                        (128, 8, 2, 32))
                    qn4 = qn.rearrange("p (h j d) -> p h j d", h=8, j=2)
                    nc.vector.tensor_mul(out=tm4[:, :, 0, :],
                                         in0=qn4[:, :, 1, :],
                                         in1=s4[:, :, 0, :])
                    nc.vector.tensor_mul(out=tm4[:, :, 1, :],
                                         in0=qn4[:, :, 0, :],
                                         in1=s4[:, :, 1, :])
                    nc.vector.tensor_mul(out=r4, in0=qn4, in1=c4)
                    nc.vector.tensor_add(out=r4, in0=r4, in1=tm4)

                    # ---- transpose to feature-major (one bank, one copy) ----
                    dstT = qTa if which == "q" else kTa
                    trans4(rot, dstT[:, :, tg * 128:(tg + 1) * 128],
                           fast_copy=QK_COPY_DVE)

                def pv_head_parcels(qc, h, ptq, ynorm):
                    """Flipped PV for one head as 5 parcels (4 subs +
                    normalize) so they can interleave with S pairs."""
                    yu_box = []

                    def sub_parcel(sub):
                        if sub == 0:
                            yu_box.append((ringp if YU_RING else yupp).tile(
                                [128, 512], F32, name="yu",
                                tag="ring" if YU_RING else "yu", bufs=YU_BUFS))
                        yu3 = yu_box[0].rearrange(
                            "p (s f) -> p s f", s=4)[:, :, 0:65]
                        last_kt = 4 * qc + sub
                        for kt in range(last_kt + 1):
                            nc.tensor.matmul(
                                yu3[:, sub, :],
                                ptq[kt // 2][:, kt % 2,
                                             sub * 128:(sub + 1) * 128],
                                vsb[kt][:, h, :],
                                start=(sub == 0 and kt == 0),
                                stop=(sub == 3 and kt == last_kt),
                                skip_group_check=True)

                    def norm_parcel():
                        yu3 = yu_box[0].rearrange(
                            "p (s f) -> p s f", s=4)[:, :, 0:65]
                        rec = p2.tile([128, 4, 1], F32, name="rec", tag="rec",
                                      bufs=2)
                        nc.vector.reciprocal(out=rec, in_=yu3[:, :, 64:65])
                        nc.vector.tensor_mul(
                            out=ynorm[:, :, h, :], in0=yu3[:, :, 0:64],
                            in1=rec.to_broadcast((128, 4, 64)))

                    return [lambda s=s: sub_parcel(s) for s in range(4)] + \
                        [norm_parcel]

                def do_p2(qc, units, pre_pv):
                    """Attention for query chunk qc; `units` are next-chunk
                    phase-1 closures woven between heads; the first `pre_pv`
                    units are emitted before head 0's PV (qc==0 needs chunk-0
                    v tiles there).  PV runs one head behind S/exp so the PE
                    never waits on the current head's exp."""
                    npair = 2 * qc + 2
                    it = iter(units)
                    ynorm = p2.tile([128, 4, 8, 64], F16, name="yn",
                                    tag="yn", bufs=2)
                    prev = None
                    for h in range(8):
                        hp, b0 = h // 2, 64 * (h % 2)
                        # fillers: previous head's PV parcels + next-chunk
                        # phase-1 units, spread between this head's S pairs
                        fillers = []
                        if prev is not None:
                            fillers.extend(
                                pv_head_parcels(qc, prev[0], prev[1], ynorm))
                        for _ in range(pre_pv if h == 0 else 2):
                            u = next(it, None)
                            if u is not None:
                                fillers.append(u)
                        fidx = [0.0]
                        fstep = len(fillers) / (4 * qc + 4) if SCHED == 2 else 0.0

                        def drain(upto):
                            while fidx[0] < upto and int(fidx[0]) < len(fillers):
                                fillers[int(fidx[0])]()
                                fidx[0] += 1.0

                        if SCHED == 1:
                            drain(len(fillers))
                        ptq = [p2.tile([128, 2, 512], F16, name="ptq",
                                       tag="ptq", bufs=4 * NQ + 2)
                               for _ in range(npair)]
                        kt_order = (list(reversed(range(4 * qc + 4)))
                                    if KT_REV else list(range(4 * qc + 4)))
                        for kt in kt_order:
                            m = kt - 4 * qc
                            a0 = max(0, 128 * m)
                            st1 = stpp.tile([128, 512], F32, name="st1",
                                            tag="st1", bufs=ST_BUFS)
                            nc.tensor.matmul(
                                st1[:, a0:512],
                                kTa[b0:b0 + 64, hp,
                                    kt * 128:(kt + 1) * 128],
                                qTa[b0:b0 + 64, hp,
                                    qc * 512 + a0:(qc + 1) * 512],
                                start=True, stop=True)
                            ptt = ptq[kt // 2][:, kt % 2, :]
                            if m < 0 and kt < {1: 2 * SCH_Q1, 2: 2 * SCH_Q2, 3: 2 * SCH_Q3}.get(qc, 0):
                                # fast exp: f16 bits = int16(A*s + B)
                                nc.vector.tensor_scalar(
                                    out=ptt.bitcast(I16), in0=st1,
                                    scalar1=SCH_A, scalar2=SCH_B,
                                    op0=mybir.AluOpType.mult,
                                    op1=mybir.AluOpType.add)
                            elif m < 0:
                                nc.scalar.activation(
                                    ptt, st1,
                                    mybir.ActivationFunctionType.Exp,
                                    scale=SCALE)
                            else:
                                w0 = 128 * m
                                nc.scalar.activation(
                                    ptt[:, w0:512], st1[:, w0:512],
                                    mybir.ActivationFunctionType.Exp,
                                    scale=SCALE)
                                (nc.vector if TRI_DVE else nc.gpsimd).tensor_mul(
                                    out=ptt[:, w0:w0 + 128],
                                    in0=ptt[:, w0:w0 + 128],
                                    in1=tri16)
                            if SCHED == 2:
                                drain((kt + 1) * fstep)

                        drain(len(fillers))
                        prev = (h, ptq)

                    for f in pv_head_parcels(qc, prev[0], prev[1], ynorm):
                        f()

                    # ---- y -> feature-major fp8; projection; store ----
                    yT8 = p2.tile([128, 4, 512], F16, name="yT8", tag="yT8",
                                  bufs=2)
                    for sub in range(4):
                        u = next(it, None)
                        if u is not None:
                            u()
                        yflat = ynorm[:, sub, :, :].rearrange(
                            "p h d -> p (h d)")
                        trans4(yflat, yT8[:, :, sub * 128:(sub + 1) * 128],
                               fast_copy=YTP_DVE)
                        for jc in range(2):
                            pr = ringp.tile([128, 512], F32, name="pr",
                                            tag="ring", bufs=2)
                            for i in range(4):
                                nc.tensor.matmul(
                                    pr,
                                    yT8[:, i, sub * 128:(sub + 1) * 128],
                                    wp_sb[:, i, jc * 512:(jc + 1) * 512],
                                    start=(i == 0), stop=(i == 3))
                            osb = p2.tile([128, 512], F16, name="osb",
                                          tag="osb", bufs=3)
                            if OSB_DVE and jc == 0:
                                nc.vector.tensor_copy(out=osb, in_=pr)
                            else:
                                nc.scalar.copy(out=osb, in_=pr)
                            r0 = qc * 512 + sub * 128
                            nc.sync.dma_start(
                                out=out_d[r0:r0 + 128,
                                          jc * 512:(jc + 1) * 512],
                                in_=osb)
                    for u in it:
                        u()

                for ii in range(NQ):
                    units = p1_units(ii)
                    if ii == 0:
                        units[0]()                 # x chunk-0 dma first
                        dma_weights_early()        # wq + rope tables
                        units[1]()                 # q ts0
                        dma_weights_late()         # wk, wv, wp
                        for u in units[2:9]:       # q*3 + k*4
                            u()
                        carry = units[9:]          # v*4
                    else:
                        do_p2(ii - 1, carry + units, pre_pv=len(carry))
                        carry = []
                do_p2(NQ - 1, [], pre_pv=0)

    _legalize_waits(nc)
    return nc


def _host_tables():
    inv_freq = 1.0 / (10000.0 ** (np.arange(0, D, 2, dtype=np.float32) / D))
    t = np.arange(T, dtype=np.float32)
    freqs = np.outer(t, inv_freq).astype(np.float32)      # (T, 32)
    cos16 = np.cos(freqs).astype(np.float16)
    s = np.sin(freqs)
    sin16 = np.concatenate([s, -s], axis=1).astype(np.float16)  # [T, 64]
    p = np.arange(128)[:, None]
    f = np.arange(128)[None, :]
    tri = (p <= f).astype(np.float16)                     # (128, 128)
    return cos16, sin16, tri


_CACHE = {}


def kernel(x, v1, wq, wk, wv, wproj, lamb):
    x = np.asarray(x, dtype=np.float32)
    v1 = np.asarray(v1, dtype=np.float32)
    wq = np.asarray(wq, dtype=np.float32)
    wk = np.asarray(wk, dtype=np.float32)
    wv = np.asarray(wv, dtype=np.float32)
    wproj = np.asarray(wproj, dtype=np.float32)
    lam = float(np.asarray(lamb))

    cosn, sinn, tri = _host_tables()

    key = lam
    if key not in _CACHE:
        _CACHE[key] = _build(lam)
    nc = _CACHE[key]

    in_maps = []
    for core in range(8):
        b, hg = core // 2, core % 2
        sl = slice(hg * HG, (hg + 1) * HG)
        in_maps.append({
            "xbT": np.ascontiguousarray(x[b].T).astype(np.float16),
            "v1b": np.ascontiguousarray(lam * v1[b][:, sl]).astype(np.float16),
            "wqT": np.ascontiguousarray(wq[sl, :].T).astype(np.float16),
            "wkT": np.ascontiguousarray(wk[sl, :].T).astype(np.float16),
            "wvT": np.ascontiguousarray(wv[sl, :].T).astype(np.float16),
            "wpT": np.ascontiguousarray(wproj[:, sl].T).astype(np.float16),
            "cosn": cosn,
            "sinn": sinn,
            "tri01": tri,
        })

    res = bass_utils.run_bass_kernel_spmd(nc, in_maps, core_ids=list(range(8)))
    y = np.empty((B, T, C), dtype=np.float32)
    for b in range(B):
        y[b] = (res.results[2 * b]["out"].astype(np.float32)
                + res.results[2 * b + 1]["out"].astype(np.float32))
    return (y, v1)
